# revision 25
# baseline (speedup 1.0000x reference)
"""nn_Decoder kernel: LSTM+attention decoder, vocab-sharded readout on 8 trn2 cores.

Strategy:
- The 32-step recurrent LSTM/attention part is tiny (~0.4 GFLOP, B=32) and
  strictly sequential; computed exactly on host in fp32.
- The readout projection logits = pre @ readout_W.T ([1024,512]@[512,32000],
  the memory-dominant part) runs on 8 NeuronCores, tensor-parallel over
  vocab (4000 cols/core): bf16 operands, fp32 PSUM accumulation, bf16 out.
"""
import numpy as np

D = 512
V = 32000
NEG_INF = 1e9
N_CORES = 8
VSH = V // N_CORES  # 4000

try:
    import ml_dtypes
    _BF16 = np.dtype(ml_dtypes.bfloat16)
except Exception:  # pragma: no cover
    _BF16 = None


def _sigmoid(x):
    return 1.0 / (1.0 + np.exp(-x))


def _recurrence(x_enc, x_enc_k, h0, c0, x_mask, y_train, word_emb, W_ih, W_hh,
                b_ih, b_hh, w_trg_W, w_trg_b, w_att_W, w_att_b, ctx2r_W):
    B, Ly = y_train.shape
    f32 = np.float32
    emb = word_emb[y_train].astype(f32)              # [B, Ly, DW]
    h = h0.astype(f32).copy()
    c = c0.astype(f32).copy()
    feed = np.zeros((B, 2 * D), f32)
    W_ih_T = np.ascontiguousarray(W_ih.T.astype(f32))
    W_hh_T = np.ascontiguousarray(W_hh.T.astype(f32))
    w_trg_T = np.ascontiguousarray(w_trg_W.T.astype(f32))
    ctx2r_T = np.ascontiguousarray(ctx2r_W.T.astype(f32))
    a = w_att_W[0].astype(f32)                       # [D]
    mask_add = np.where(x_mask, f32(-NEG_INF), f32(0.0))  # [B,Lx]
    pre_all = np.empty((Ly, B, D), f32)
    for t in range(Ly):
        x = np.concatenate([emb[:, t, :], feed], axis=1)       # [B, DW+2D]
        gates = x @ W_ih_T + b_ih + h @ W_hh_T + b_hh
        i, f, g, o = np.split(gates, 4, axis=1)
        c = _sigmoid(f) * c + _sigmoid(i) * np.tanh(g)
        h = _sigmoid(o) * np.tanh(c)
        q = h @ w_trg_T + w_trg_b                              # [B, D]
        att = np.tanh(x_enc_k + q[:, None, :])                 # [B, Lx, D]
        scores = att @ a + w_att_b[0] + mask_add               # [B, Lx]
        scores = scores - scores.max(axis=1, keepdims=True)
        e = np.exp(scores)
        w = e / e.sum(axis=1, keepdims=True)
        ctx = np.einsum("bl,bld->bd", w, x_enc).astype(f32)    # [B, 2D]
        feed = ctx
        pre_all[t] = np.tanh(np.concatenate([h, ctx], axis=1) @ ctx2r_T)
    return pre_all                                              # [Ly, B, D]


_BASS_CACHE = {}


def _build_bass_matmul():
    """SPMD kernel: out[1024, 4000] = preT[512,1024].T @ wT[512,4000], bf16 in."""
    import concourse.bacc as bacc
    import concourse.tile as tile
    from concourse import mybir

    # Bacc (not raw Bass): its compile() pass splits multi-semaphore waits
    # into event-semaphore chains - TRN2 allows at most 1 wait/instruction.
    nc = bacc.Bacc("TRN2", target_bir_lowering=False)
    f32 = mybir.dt.float32
    bf16 = mybir.dt.bfloat16
    preT = nc.declare_dram_parameter("preT", [512, 1024], bf16, isOutput=False)
    wT = nc.declare_dram_parameter("wT", [512, VSH], bf16, isOutput=False)
    out = nc.declare_dram_parameter("out", [1024, VSH], bf16, isOutput=True)

    N_WARMUP = 10
    preR = preT.rearrange("(k p) c -> p k c", k=4)   # k-major views: one DMA
    wR = wT.rearrange("(k p) c -> p k c", k=4)       # loads all 4 k-slices

    with tile.TileContext(nc) as tc:
        with tc.tile_pool(name="weights", bufs=1) as wpool, \
             tc.tile_pool(name="psum", bufs=8, space="PSUM") as ppool, \
             tc.tile_pool(name="outs", bufs=8) as opool:
            # PE p-state warmup: the tensor engine ramps 0.65->1.2->2.4GHz
            # over ~3us of continuous execution. Dummy matmuls during the
            # weight-fill window absorb the ramp so the real matmuls run at
            # full clock; the warmup is sized to end right when the first
            # weights land, since an idle gap would reset the ramp.
            wu = wpool.tile([128, 500], bf16, tag="warmup_src")
            nc.vector.memset(wu[:, 0:8], 0.0)
            wups = ppool.tile([8, 500], f32, tag="ps")
            for _ in range(N_WARMUP):
                nc.tensor.matmul(wups, wu[:, 0:8], wu, start=True, stop=True)

            # Inputs: one k-major DMA per logical block, issued in first-use
            # order. Few, large DMAs keep the serialized descriptor-generation
            # queues short (HWDGE 625ns/DMA; Pool/SWDGE ~1.3us/DMA runs in
            # parallel with HWDGE, so preT rides the Pool queue).
            wtile = [None] * 8                       # [j] -> [128,4,500]
            wc0 = wpool.tile([128, 4, 500], bf16, tag="wc0")
            wtile[0] = wc0
            nc.sync.dma_start(out=wc0, in_=wR[:, :, 0:500])
            pm0 = wpool.tile([128, 4, 128], bf16, tag="preT_m0")
            nc.sync.dma_start(out=pm0, in_=preR[:, :, 0:128])
            ph0 = wpool.tile([128, 4, 384], bf16, tag="preT_h0")
            nc.sync.dma_start(out=ph0, in_=preR[:, :, 128:512])
            ph1 = wpool.tile([128, 4, 512], bf16, tag="preT_h1")
            nc.sync.dma_start(out=ph1, in_=preR[:, :, 512:1024])
            for j in range(1, 8):
                wcj = wpool.tile([128, 4, 500], bf16, tag=f"wc{j}")
                wtile[j] = wcj
                nc.sync.dma_start(out=wcj,
                                  in_=wR[:, :, j * 500:(j + 1) * 500])

            def pre_view(m, k):
                if m == 0:
                    return pm0[:, k, :]
                if m <= 3:
                    return ph0[:, k, (m - 1) * 128:m * 128]
                return ph1[:, k, (m - 4) * 128:(m - 3) * 128]

            def w_view(k, col, width):
                j, off = divmod(col, 500)
                return wtile[j][:, k, off:off + width]

            # psum chunk schedule: (col, width). Early chunks pair into
            # [128,1000] staging tiles (fewer HWDGE descriptor gens); the
            # last two stay single so their output DMAs spread over the
            # final sweeps instead of bursting after the last matmul.
            chunks = [(j * 500, 500) for j in range(8)]
            # output staging: (start_col, width, [chunk idxs])
            groups = [(0, 1000, [0, 1]), (1000, 1000, [2, 3]),
                      (2000, 1000, [4, 5]), (3000, 500, [6]),
                      (3500, 500, [7])]
            grp_of = {}
            for gi, (gcol, gw, cidx) in enumerate(groups):
                for ci in cidx:
                    grp_of[ci] = (gi, gcol, gw, cidx[-1])

            otiles = {}
            for ci, (col, width) in enumerate(chunks):
                gi, gcol, gw, last_ci = grp_of[ci]
                for m in range(8):
                    if ci == 7 and m == 7:
                        # very last group: [375]+[125] sub-chunks so the
                        # post-last-matmul copy+DMA chain handles only 125
                        # columns; the [375] part ships via the idle Pool
                        # DGE queue in parallel.
                        psa = ppool.tile([128, 375], f32, tag="ps")
                        for k in range(4):
                            nc.tensor.matmul(
                                psa, pre_view(m, k), w_view(k, col, 375),
                                start=(k == 0), stop=(k == 3))
                        ota = opool.tile([128, 375], bf16, tag="otla")
                        nc.scalar.copy(ota, psa)
                        nc.gpsimd.dma_start(
                            out=out[m * 128:(m + 1) * 128, col:col + 375],
                            in_=ota)
                        psb = ppool.tile([128, 125], f32, tag="ps")
                        for k in range(4):
                            nc.tensor.matmul(
                                psb, pre_view(m, k),
                                w_view(k, col + 375, 125),
                                start=(k == 0), stop=(k == 3))
                        otb = opool.tile([128, 125], bf16, tag="otlb")
                        nc.vector.tensor_copy(otb, psb)
                        nc.sync.dma_start(
                            out=out[m * 128:(m + 1) * 128,
                                    col + 375:col + 500],
                            in_=otb)
                        continue
                    ps = ppool.tile([128, width], f32, tag="ps")
                    for k in range(4):
                        nc.tensor.matmul(
                            ps, pre_view(m, k), w_view(k, col, width),
                            start=(k == 0), stop=(k == 3))
                    if (gi, m) not in otiles:
                        ot_new = opool.tile([128, gw], bf16,
                                            tag=f"ot{gi % 3}_{m % 2}")
                        otiles[(gi, m)] = ot_new
                    ot = otiles[(gi, m)]
                    dst = ot[:, col - gcol:col - gcol + width]
                    if (ci + m) % 2 == 0:
                        nc.scalar.copy(dst, ps)
                    else:
                        nc.vector.tensor_copy(dst, ps)
                    if ci == last_ci:
                        nc.sync.dma_start(
                            out=out[m * 128:(m + 1) * 128, gcol:gcol + gw],
                            in_=ot)
    nc.finalize()
    return nc


def _install_neff_cache():
    """Persistent NEFF cache for the bass_exec compile path.

    concourse's neuronx_cc_hook compiles the embedded BIR with walrus on
    every fresh process (60-190s) and never consults libneuronxla's NEFF
    cache. The BIR built by _build_bass_matmul is byte-deterministic, so a
    content-addressed on-disk cache makes every process after the first
    skip the compile entirely.
    """
    import hashlib
    import os
    import shutil
    import concourse.bass2jax as b2j
    if getattr(b2j, "_bass_neff_cache_installed", False):
        return
    orig = b2j.compile_bir_kernel
    cache_root = os.environ.get(
        "BASS_NEFF_CACHE_DIR",
        os.path.expanduser("~/.neuron-compile-cache/bass-neff"))

    def cached_compile(bir_json, tmpdir, neff_name="file.neff"):
        try:
            raw = bir_json if isinstance(bir_json, bytes) else bir_json.encode()
            key = hashlib.sha256(raw).hexdigest()
            cpath = os.path.join(cache_root, key + ".neff")
            if os.path.exists(cpath):
                dst = os.path.join(tmpdir, neff_name)
                shutil.copyfile(cpath, dst)
                return dst
            neff_path = orig(bir_json, tmpdir, neff_name=neff_name)
            os.makedirs(cache_root, exist_ok=True)
            tmp = cpath + ".tmp"
            shutil.copyfile(neff_path, tmp)
            os.replace(tmp, cpath)
            return neff_path
        except Exception:
            return orig(bir_json, tmpdir, neff_name=neff_name)

    b2j.compile_bir_kernel = cached_compile
    b2j._bass_neff_cache_installed = True


def _readout_device(pre_flat, wT_bf16):
    """pre_flat [1024, 512] fp32 -> logits [1024, 32000] via 8-core bass."""
    import os
    from concourse.bass_utils import run_bass_kernel_spmd
    _install_neff_cache()
    if "nc" not in _BASS_CACHE:
        _BASS_CACHE["nc"] = _build_bass_matmul()
    nc = _BASS_CACHE["nc"]
    preT = np.ascontiguousarray(pre_flat.T).astype(_BF16)    # [512, 1024] bf16
    in_maps = [
        {"preT": preT, "wT": np.ascontiguousarray(wT_bf16[:, k * VSH:(k + 1) * VSH])}
        for k in range(N_CORES)
    ]
    core_ids = list(range(N_CORES))
    try:
        res = run_bass_kernel_spmd(nc, in_maps, core_ids=core_ids)
    except Exception:
        # e.g. BASS_TRACE set but the axon NTFF profile hook is not
        # available in this container: retry with tracing disabled
        os.environ["BASS_NEVER_TRACE"] = "1"
        res = run_bass_kernel_spmd(nc, in_maps, core_ids=core_ids)
    _BASS_CACHE["last_exec_ns"] = res.exec_time_ns
    _BASS_CACHE["last_results"] = res
    return np.concatenate(
        [r["out"].astype(np.float32) for r in res.results], axis=1)


def kernel(x_enc, x_enc_k, h0, c0, x_mask, y_train, word_emb, W_ih, W_hh,
           b_ih, b_hh, w_trg_W, w_trg_b, w_att_W, w_att_b, ctx2r_W, readout_W):
    x_enc = np.asarray(x_enc, np.float32)
    x_enc_k = np.asarray(x_enc_k, np.float32)
    y_train = np.asarray(y_train)
    B, Ly = y_train.shape
    pre_all = _recurrence(x_enc, x_enc_k, np.asarray(h0), np.asarray(c0),
                          np.asarray(x_mask), y_train, np.asarray(word_emb),
                          np.asarray(W_ih), np.asarray(W_hh), np.asarray(b_ih),
                          np.asarray(b_hh), np.asarray(w_trg_W),
                          np.asarray(w_trg_b), np.asarray(w_att_W),
                          np.asarray(w_att_b), np.asarray(ctx2r_W))
    pre_flat = pre_all.reshape(Ly * B, D)                # [1024, 512]
    wT = np.ascontiguousarray(np.asarray(readout_W, np.float32).T)  # [512, V]
    try:
        if _BF16 is None:
            raise RuntimeError("ml_dtypes unavailable")
        logits_flat = _readout_device(pre_flat, wT.astype(_BF16))
        # spot-check a few token rows against the fp32 host matmul; a
        # healthy bf16 device result sits at ~4e-3 relative error, so 1.5e-2
        # catches any transient bad execution well before the 2e-2 gate
        idx = [0, 401, 702, 1023]
        ref_rows = pre_flat[idx] @ wT
        spot = np.abs(logits_flat[idx] - ref_rows).max() / max(
            np.abs(ref_rows).max(), 1e-12)
        if not np.isfinite(spot) or spot > 1.5e-2:
            raise RuntimeError(f"device spot-check failed (rel {spot:.3e})")
    except Exception as exc:                             # robust fallback
        import traceback
        traceback.print_exc()
        print(f"[kernel] device readout failed ({exc!r}); numpy fallback")
        _BASS_CACHE.pop("last_exec_ns", None)
        _BASS_CACHE.pop("last_results", None)
        logits_flat = pre_flat @ wT
    logits = logits_flat.reshape(Ly, B, V)
    return np.swapaxes(logits, 0, 1).astype(np.float32)  # [B, Ly, V]


# revision 36
# speedup vs baseline: 1.0033x; 1.0033x over previous
"""nn_Decoder kernel: LSTM+attention decoder, vocab-sharded readout on 8 trn2 cores.

Strategy:
- The 32-step recurrent LSTM/attention part is tiny (~0.4 GFLOP, B=32) and
  strictly sequential; computed exactly on host in fp32.
- The readout projection logits = pre @ readout_W.T ([1024,512]@[512,32000],
  the memory-dominant part) runs on 8 NeuronCores, tensor-parallel over
  vocab (4000 cols/core): bf16 operands, fp32 PSUM accumulation, bf16 out.
"""
import numpy as np

D = 512
V = 32000
NEG_INF = 1e9
N_CORES = 8
VSH = V // N_CORES  # 4000

try:
    import ml_dtypes
    _BF16 = np.dtype(ml_dtypes.bfloat16)
except Exception:  # pragma: no cover
    _BF16 = None


def _sigmoid(x):
    return 1.0 / (1.0 + np.exp(-x))


def _recurrence(x_enc, x_enc_k, h0, c0, x_mask, y_train, word_emb, W_ih, W_hh,
                b_ih, b_hh, w_trg_W, w_trg_b, w_att_W, w_att_b, ctx2r_W):
    B, Ly = y_train.shape
    f32 = np.float32
    emb = word_emb[y_train].astype(f32)              # [B, Ly, DW]
    h = h0.astype(f32).copy()
    c = c0.astype(f32).copy()
    feed = np.zeros((B, 2 * D), f32)
    W_ih_T = np.ascontiguousarray(W_ih.T.astype(f32))
    W_hh_T = np.ascontiguousarray(W_hh.T.astype(f32))
    w_trg_T = np.ascontiguousarray(w_trg_W.T.astype(f32))
    ctx2r_T = np.ascontiguousarray(ctx2r_W.T.astype(f32))
    a = w_att_W[0].astype(f32)                       # [D]
    mask_add = np.where(x_mask, f32(-NEG_INF), f32(0.0))  # [B,Lx]
    pre_all = np.empty((Ly, B, D), f32)
    for t in range(Ly):
        x = np.concatenate([emb[:, t, :], feed], axis=1)       # [B, DW+2D]
        gates = x @ W_ih_T + b_ih + h @ W_hh_T + b_hh
        i, f, g, o = np.split(gates, 4, axis=1)
        c = _sigmoid(f) * c + _sigmoid(i) * np.tanh(g)
        h = _sigmoid(o) * np.tanh(c)
        q = h @ w_trg_T + w_trg_b                              # [B, D]
        att = np.tanh(x_enc_k + q[:, None, :])                 # [B, Lx, D]
        scores = att @ a + w_att_b[0] + mask_add               # [B, Lx]
        scores = scores - scores.max(axis=1, keepdims=True)
        e = np.exp(scores)
        w = e / e.sum(axis=1, keepdims=True)
        ctx = np.einsum("bl,bld->bd", w, x_enc).astype(f32)    # [B, 2D]
        feed = ctx
        pre_all[t] = np.tanh(np.concatenate([h, ctx], axis=1) @ ctx2r_T)
    return pre_all                                              # [Ly, B, D]


_BASS_CACHE = {}


def _build_bass_matmul():
    """SPMD kernel: out[1024, 4000] = preT[512,1024].T @ wT[512,4000], bf16 in."""
    import concourse.bacc as bacc
    import concourse.tile as tile
    from concourse import mybir

    # Bacc (not raw Bass): its compile() pass splits multi-semaphore waits
    # into event-semaphore chains - TRN2 allows at most 1 wait/instruction.
    nc = bacc.Bacc("TRN2", target_bir_lowering=False)
    f32 = mybir.dt.float32
    bf16 = mybir.dt.bfloat16
    preT = nc.declare_dram_parameter("preT", [512, 1024], bf16, isOutput=False)
    wT = nc.declare_dram_parameter("wT", [512, VSH], bf16, isOutput=False)
    out = nc.declare_dram_parameter("out", [1024, VSH], bf16, isOutput=True)

    N_WARMUP = 10
    preR = preT.rearrange("(k p) c -> p k c", k=4)   # k-major views: one DMA
    wR = wT.rearrange("(k p) c -> p k c", k=4)       # loads all 4 k-slices

    with tile.TileContext(nc) as tc:
        with tc.tile_pool(name="weights", bufs=1) as wpool, \
             tc.tile_pool(name="psum", bufs=8, space="PSUM") as ppool, \
             tc.tile_pool(name="outs", bufs=8) as opool:
            # PE p-state warmup: the tensor engine ramps 0.65->1.2->2.4GHz
            # over ~3us of continuous execution. Dummy matmuls during the
            # weight-fill window absorb the ramp so the real matmuls run at
            # full clock; the warmup is sized to end right when the first
            # weights land, since an idle gap would reset the ramp.
            wu = wpool.tile([128, 500], bf16, tag="warmup_src")
            nc.vector.memset(wu[:, 0:8], 0.0)
            wups = ppool.tile([8, 500], f32, tag="ps")
            for _ in range(N_WARMUP - 1):
                nc.tensor.matmul(wups, wu[:, 0:8], wu, start=True, stop=True)
            nc.tensor.matmul(wups[:, 0:250], wu[:, 0:8], wu[:, 0:250],
                             start=True, stop=True)

            # Inputs: one k-major DMA per logical block, issued in first-use
            # order. Few, large DMAs keep the serialized descriptor-generation
            # queues short (HWDGE 625ns/DMA; Pool/SWDGE ~1.3us/DMA runs in
            # parallel with HWDGE, so preT rides the Pool queue).
            wtile = [None] * 8                       # [j] -> [128,4,500]
            wc0 = wpool.tile([128, 4, 500], bf16, tag="wc0")
            wtile[0] = wc0
            nc.sync.dma_start(out=wc0, in_=wR[:, :, 0:500])
            pm0 = wpool.tile([128, 4, 128], bf16, tag="preT_m0")
            nc.sync.dma_start(out=pm0, in_=preR[:, :, 0:128])
            ph0 = wpool.tile([128, 4, 384], bf16, tag="preT_h0")
            nc.sync.dma_start(out=ph0, in_=preR[:, :, 128:512])
            ph1 = wpool.tile([128, 4, 512], bf16, tag="preT_h1")
            nc.sync.dma_start(out=ph1, in_=preR[:, :, 512:1024])
            for j in range(1, 8):
                wcj = wpool.tile([128, 4, 500], bf16, tag=f"wc{j}")
                wtile[j] = wcj
                nc.sync.dma_start(out=wcj,
                                  in_=wR[:, :, j * 500:(j + 1) * 500])

            def pre_view(m, k):
                if m == 0:
                    return pm0[:, k, :]
                if m <= 3:
                    return ph0[:, k, (m - 1) * 128:m * 128]
                return ph1[:, k, (m - 4) * 128:(m - 3) * 128]

            def w_view(k, col, width):
                j, off = divmod(col, 500)
                return wtile[j][:, k, off:off + width]

            # psum chunk schedule: (col, width). Early chunks pair into
            # [128,1000] staging tiles (fewer HWDGE descriptor gens); the
            # last two stay single so their output DMAs spread over the
            # final sweeps instead of bursting after the last matmul.
            chunks = [(j * 500, 500) for j in range(8)]
            # output staging: (start_col, width, [chunk idxs])
            groups = [(0, 1000, [0, 1]), (1000, 1000, [2, 3]),
                      (2000, 1000, [4, 5]), (3000, 500, [6]),
                      (3500, 500, [7])]
            grp_of = {}
            for gi, (gcol, gw, cidx) in enumerate(groups):
                for ci in cidx:
                    grp_of[ci] = (gi, gcol, gw, cidx[-1])

            otiles = {}
            for ci, (col, width) in enumerate(chunks):
                gi, gcol, gw, last_ci = grp_of[ci]
                for m in range(8):
                    if ci == 7 and m == 7:
                        # very last group: [400]+[100] sub-chunks with the
                        # copies on different engines and the DMAs on
                        # different DGE queues, so the two post-last-matmul
                        # chains drain in parallel.
                        psa = ppool.tile([128, 400], f32, tag="ps")
                        for k in range(4):
                            nc.tensor.matmul(
                                psa, pre_view(m, k), w_view(k, col, 400),
                                start=(k == 0), stop=(k == 3))
                        ota = opool.tile([128, 400], bf16, tag="otla")
                        nc.scalar.copy(ota, psa)
                        nc.sync.dma_start(
                            out=out[m * 128:(m + 1) * 128, col:col + 400],
                            in_=ota)
                        psb = ppool.tile([128, 100], f32, tag="ps")
                        for k in range(4):
                            nc.tensor.matmul(
                                psb, pre_view(m, k),
                                w_view(k, col + 400, 100),
                                start=(k == 0), stop=(k == 3))
                        otb = opool.tile([128, 100], bf16, tag="otlb")
                        nc.vector.tensor_copy(otb, psb)
                        nc.gpsimd.dma_start(
                            out=out[m * 128:(m + 1) * 128,
                                    col + 400:col + 500],
                            in_=otb)
                        continue
                    ps = ppool.tile([128, width], f32, tag="ps")
                    for k in range(4):
                        nc.tensor.matmul(
                            ps, pre_view(m, k), w_view(k, col, width),
                            start=(k == 0), stop=(k == 3))
                    if (gi, m) not in otiles:
                        ot_new = opool.tile([128, gw], bf16,
                                            tag=f"ot{gi % 3}_{m % 2}")
                        otiles[(gi, m)] = ot_new
                    ot = otiles[(gi, m)]
                    dst = ot[:, col - gcol:col - gcol + width]
                    if (ci + m) % 2 == 0:
                        nc.scalar.copy(dst, ps)
                    else:
                        nc.vector.tensor_copy(dst, ps)
                    if ci == last_ci:
                        nc.sync.dma_start(
                            out=out[m * 128:(m + 1) * 128, gcol:gcol + gw],
                            in_=ot)
    nc.finalize()
    return nc


def _install_neff_cache():
    """Persistent NEFF cache for the bass_exec compile path.

    concourse's neuronx_cc_hook compiles the embedded BIR with walrus on
    every fresh process (60-190s) and never consults libneuronxla's NEFF
    cache. The BIR built by _build_bass_matmul is byte-deterministic, so a
    content-addressed on-disk cache makes every process after the first
    skip the compile entirely.
    """
    import hashlib
    import os
    import shutil
    import concourse.bass2jax as b2j
    if getattr(b2j, "_bass_neff_cache_installed", False):
        return
    orig = b2j.compile_bir_kernel
    cache_root = os.environ.get(
        "BASS_NEFF_CACHE_DIR",
        os.path.expanduser("~/.neuron-compile-cache/bass-neff"))

    def cached_compile(bir_json, tmpdir, neff_name="file.neff"):
        try:
            raw = bir_json if isinstance(bir_json, bytes) else bir_json.encode()
            key = hashlib.sha256(raw).hexdigest()
            cpath = os.path.join(cache_root, key + ".neff")
            if os.path.exists(cpath):
                dst = os.path.join(tmpdir, neff_name)
                shutil.copyfile(cpath, dst)
                return dst
            neff_path = orig(bir_json, tmpdir, neff_name=neff_name)
            os.makedirs(cache_root, exist_ok=True)
            tmp = cpath + ".tmp"
            shutil.copyfile(neff_path, tmp)
            os.replace(tmp, cpath)
            return neff_path
        except Exception:
            return orig(bir_json, tmpdir, neff_name=neff_name)

    b2j.compile_bir_kernel = cached_compile
    b2j._bass_neff_cache_installed = True


def _readout_device(pre_flat, wT_bf16):
    """pre_flat [1024, 512] fp32 -> logits [1024, 32000] via 8-core bass."""
    import os
    from concourse.bass_utils import run_bass_kernel_spmd
    _install_neff_cache()
    if "nc" not in _BASS_CACHE:
        _BASS_CACHE["nc"] = _build_bass_matmul()
    nc = _BASS_CACHE["nc"]
    preT = np.ascontiguousarray(pre_flat.T).astype(_BF16)    # [512, 1024] bf16
    in_maps = [
        {"preT": preT, "wT": np.ascontiguousarray(wT_bf16[:, k * VSH:(k + 1) * VSH])}
        for k in range(N_CORES)
    ]
    core_ids = list(range(N_CORES))
    try:
        res = run_bass_kernel_spmd(nc, in_maps, core_ids=core_ids)
    except Exception:
        # e.g. BASS_TRACE set but the axon NTFF profile hook is not
        # available in this container: retry with tracing disabled
        os.environ["BASS_NEVER_TRACE"] = "1"
        res = run_bass_kernel_spmd(nc, in_maps, core_ids=core_ids)
    _BASS_CACHE["last_exec_ns"] = res.exec_time_ns
    _BASS_CACHE["last_results"] = res
    return np.concatenate(
        [r["out"].astype(np.float32) for r in res.results], axis=1)


def kernel(x_enc, x_enc_k, h0, c0, x_mask, y_train, word_emb, W_ih, W_hh,
           b_ih, b_hh, w_trg_W, w_trg_b, w_att_W, w_att_b, ctx2r_W, readout_W):
    x_enc = np.asarray(x_enc, np.float32)
    x_enc_k = np.asarray(x_enc_k, np.float32)
    y_train = np.asarray(y_train)
    B, Ly = y_train.shape
    pre_all = _recurrence(x_enc, x_enc_k, np.asarray(h0), np.asarray(c0),
                          np.asarray(x_mask), y_train, np.asarray(word_emb),
                          np.asarray(W_ih), np.asarray(W_hh), np.asarray(b_ih),
                          np.asarray(b_hh), np.asarray(w_trg_W),
                          np.asarray(w_trg_b), np.asarray(w_att_W),
                          np.asarray(w_att_b), np.asarray(ctx2r_W))
    pre_flat = pre_all.reshape(Ly * B, D)                # [1024, 512]
    wT = np.ascontiguousarray(np.asarray(readout_W, np.float32).T)  # [512, V]
    try:
        if _BF16 is None:
            raise RuntimeError("ml_dtypes unavailable")
        logits_flat = _readout_device(pre_flat, wT.astype(_BF16))
        # spot-check a few token rows against the fp32 host matmul; a
        # healthy bf16 device result sits at ~4e-3 relative error, so 1.5e-2
        # catches any transient bad execution well before the 2e-2 gate
        idx = [0, 401, 702, 1023]
        ref_rows = pre_flat[idx] @ wT
        spot = np.abs(logits_flat[idx] - ref_rows).max() / max(
            np.abs(ref_rows).max(), 1e-12)
        if not np.isfinite(spot) or spot > 1.5e-2:
            raise RuntimeError(f"device spot-check failed (rel {spot:.3e})")
    except Exception as exc:                             # robust fallback
        import traceback
        traceback.print_exc()
        print(f"[kernel] device readout failed ({exc!r}); numpy fallback")
        _BASS_CACHE.pop("last_exec_ns", None)
        _BASS_CACHE.pop("last_results", None)
        logits_flat = pre_flat @ wT
    logits = logits_flat.reshape(Ly, B, V)
    return np.swapaxes(logits, 0, 1).astype(np.float32)  # [B, Ly, V]


# revision 37
# speedup vs baseline: 1.0058x; 1.0025x over previous
"""nn_Decoder kernel: LSTM+attention decoder, vocab-sharded readout on 8 trn2 cores.

Strategy:
- The 32-step recurrent LSTM/attention part is tiny (~0.4 GFLOP, B=32) and
  strictly sequential; computed exactly on host in fp32.
- The readout projection logits = pre @ readout_W.T ([1024,512]@[512,32000],
  the memory-dominant part) runs on 8 NeuronCores, tensor-parallel over
  vocab (4000 cols/core): bf16 operands, fp32 PSUM accumulation, bf16 out.
"""
import numpy as np

D = 512
V = 32000
NEG_INF = 1e9
N_CORES = 8
VSH = V // N_CORES  # 4000

try:
    import ml_dtypes
    _BF16 = np.dtype(ml_dtypes.bfloat16)
except Exception:  # pragma: no cover
    _BF16 = None


def _sigmoid(x):
    return 1.0 / (1.0 + np.exp(-x))


def _recurrence(x_enc, x_enc_k, h0, c0, x_mask, y_train, word_emb, W_ih, W_hh,
                b_ih, b_hh, w_trg_W, w_trg_b, w_att_W, w_att_b, ctx2r_W):
    B, Ly = y_train.shape
    f32 = np.float32
    emb = word_emb[y_train].astype(f32)              # [B, Ly, DW]
    h = h0.astype(f32).copy()
    c = c0.astype(f32).copy()
    feed = np.zeros((B, 2 * D), f32)
    W_ih_T = np.ascontiguousarray(W_ih.T.astype(f32))
    W_hh_T = np.ascontiguousarray(W_hh.T.astype(f32))
    w_trg_T = np.ascontiguousarray(w_trg_W.T.astype(f32))
    ctx2r_T = np.ascontiguousarray(ctx2r_W.T.astype(f32))
    a = w_att_W[0].astype(f32)                       # [D]
    mask_add = np.where(x_mask, f32(-NEG_INF), f32(0.0))  # [B,Lx]
    pre_all = np.empty((Ly, B, D), f32)
    for t in range(Ly):
        x = np.concatenate([emb[:, t, :], feed], axis=1)       # [B, DW+2D]
        gates = x @ W_ih_T + b_ih + h @ W_hh_T + b_hh
        i, f, g, o = np.split(gates, 4, axis=1)
        c = _sigmoid(f) * c + _sigmoid(i) * np.tanh(g)
        h = _sigmoid(o) * np.tanh(c)
        q = h @ w_trg_T + w_trg_b                              # [B, D]
        att = np.tanh(x_enc_k + q[:, None, :])                 # [B, Lx, D]
        scores = att @ a + w_att_b[0] + mask_add               # [B, Lx]
        scores = scores - scores.max(axis=1, keepdims=True)
        e = np.exp(scores)
        w = e / e.sum(axis=1, keepdims=True)
        ctx = np.einsum("bl,bld->bd", w, x_enc).astype(f32)    # [B, 2D]
        feed = ctx
        pre_all[t] = np.tanh(np.concatenate([h, ctx], axis=1) @ ctx2r_T)
    return pre_all                                              # [Ly, B, D]


_BASS_CACHE = {}


def _build_bass_matmul():
    """SPMD kernel: out[1024, 4000] = preT[512,1024].T @ wT[512,4000], bf16 in."""
    import concourse.bacc as bacc
    import concourse.tile as tile
    from concourse import mybir

    # Bacc (not raw Bass): its compile() pass splits multi-semaphore waits
    # into event-semaphore chains - TRN2 allows at most 1 wait/instruction.
    nc = bacc.Bacc("TRN2", target_bir_lowering=False)
    f32 = mybir.dt.float32
    bf16 = mybir.dt.bfloat16
    preT = nc.declare_dram_parameter("preT", [512, 1024], bf16, isOutput=False)
    wT = nc.declare_dram_parameter("wT", [512, VSH], bf16, isOutput=False)
    out = nc.declare_dram_parameter("out", [1024, VSH], bf16, isOutput=True)

    N_WARMUP = 10
    preR = preT.rearrange("(k p) c -> p k c", k=4)   # k-major views: one DMA
    wR = wT.rearrange("(k p) c -> p k c", k=4)       # loads all 4 k-slices

    with tile.TileContext(nc) as tc:
        with tc.tile_pool(name="weights", bufs=1) as wpool, \
             tc.tile_pool(name="psum", bufs=8, space="PSUM") as ppool, \
             tc.tile_pool(name="outs", bufs=8) as opool:
            # PE p-state warmup: the tensor engine ramps 0.65->1.2->2.4GHz
            # over ~3us of continuous execution. Dummy matmuls during the
            # weight-fill window absorb the ramp so the real matmuls run at
            # full clock; the warmup is sized to end right when the first
            # weights land, since an idle gap would reset the ramp.
            wu = wpool.tile([128, 500], bf16, tag="warmup_src")
            nc.vector.memset(wu[:, 0:8], 0.0)
            wups = ppool.tile([8, 500], f32, tag="ps")
            for _ in range(N_WARMUP - 1):
                nc.tensor.matmul(wups, wu[:, 0:8], wu, start=True, stop=True)
            nc.tensor.matmul(wups[:, 0:250], wu[:, 0:8], wu[:, 0:250],
                             start=True, stop=True)

            # Inputs: one k-major DMA per logical block, issued in first-use
            # order. Few, large DMAs keep the serialized descriptor-generation
            # queues short (HWDGE 625ns/DMA; Pool/SWDGE ~1.3us/DMA runs in
            # parallel with HWDGE, so preT rides the Pool queue).
            wtile = [None] * 8                       # [j] -> [128,4,500]
            wc0 = wpool.tile([128, 4, 500], bf16, tag="wc0")
            wtile[0] = wc0
            nc.sync.dma_start(out=wc0, in_=wR[:, :, 0:500])
            pm0 = wpool.tile([128, 4, 128], bf16, tag="preT_m0")
            nc.sync.dma_start(out=pm0, in_=preR[:, :, 0:128])
            ph0 = wpool.tile([128, 4, 384], bf16, tag="preT_h0")
            nc.sync.dma_start(out=ph0, in_=preR[:, :, 128:512])
            ph1 = wpool.tile([128, 4, 512], bf16, tag="preT_h1")
            nc.sync.dma_start(out=ph1, in_=preR[:, :, 512:1024])
            for j in range(1, 8):
                wcj = wpool.tile([128, 4, 500], bf16, tag=f"wc{j}")
                wtile[j] = wcj
                nc.sync.dma_start(out=wcj,
                                  in_=wR[:, :, j * 500:(j + 1) * 500])

            def pre_view(m, k):
                if m == 0:
                    return pm0[:, k, :]
                if m <= 3:
                    return ph0[:, k, (m - 1) * 128:m * 128]
                return ph1[:, k, (m - 4) * 128:(m - 3) * 128]

            def w_view(k, col, width):
                j, off = divmod(col, 500)
                return wtile[j][:, k, off:off + width]

            # psum chunk schedule: (col, width). Early chunks pair into
            # [128,1000] staging tiles (fewer HWDGE descriptor gens); the
            # last two stay single so their output DMAs spread over the
            # final sweeps instead of bursting after the last matmul.
            chunks = [(j * 500, 500) for j in range(8)]
            # output staging: (start_col, width, [chunk idxs])
            groups = [(0, 1000, [0, 1]), (1000, 1000, [2, 3]),
                      (2000, 1000, [4, 5]), (3000, 500, [6]),
                      (3500, 500, [7])]
            grp_of = {}
            for gi, (gcol, gw, cidx) in enumerate(groups):
                for ci in cidx:
                    grp_of[ci] = (gi, gcol, gw, cidx[-1])

            otiles = {}
            for ci, (col, width) in enumerate(chunks):
                gi, gcol, gw, last_ci = grp_of[ci]
                for m in range(8):
                    if ci == 0 and m == 0:
                        # first group split into small sub-chunks: the PE
                        # ramp's mid-clock window right after the warmup
                        # covers cheap narrow matmuls instead of [500]s
                        g0 = opool.tile([128, 1000], bf16, tag="ot0_0")
                        otiles[(0, 0)] = g0
                        off = 0
                        for w in (100, 150, 250):
                            ps0 = ppool.tile([128, w], f32, tag="ps")
                            for k in range(4):
                                nc.tensor.matmul(
                                    ps0, pre_view(m, k), w_view(k, off, w),
                                    start=(k == 0), stop=(k == 3))
                            if off == 0:
                                nc.scalar.copy(g0[:, off:off + w], ps0)
                            else:
                                nc.vector.tensor_copy(g0[:, off:off + w], ps0)
                            off += w
                        continue
                    if ci == 7 and m == 7:
                        # very last group: [400]+[100] sub-chunks with the
                        # copies on different engines and the DMAs on
                        # different DGE queues, so the two post-last-matmul
                        # chains drain in parallel.
                        psa = ppool.tile([128, 400], f32, tag="ps")
                        for k in range(4):
                            nc.tensor.matmul(
                                psa, pre_view(m, k), w_view(k, col, 400),
                                start=(k == 0), stop=(k == 3))
                        ota = opool.tile([128, 400], bf16, tag="otla")
                        nc.scalar.copy(ota, psa)
                        nc.sync.dma_start(
                            out=out[m * 128:(m + 1) * 128, col:col + 400],
                            in_=ota)
                        psb = ppool.tile([128, 100], f32, tag="ps")
                        for k in range(4):
                            nc.tensor.matmul(
                                psb, pre_view(m, k),
                                w_view(k, col + 400, 100),
                                start=(k == 0), stop=(k == 3))
                        otb = opool.tile([128, 100], bf16, tag="otlb")
                        nc.vector.tensor_copy(otb, psb)
                        nc.gpsimd.dma_start(
                            out=out[m * 128:(m + 1) * 128,
                                    col + 400:col + 500],
                            in_=otb)
                        continue
                    ps = ppool.tile([128, width], f32, tag="ps")
                    for k in range(4):
                        nc.tensor.matmul(
                            ps, pre_view(m, k), w_view(k, col, width),
                            start=(k == 0), stop=(k == 3))
                    if (gi, m) not in otiles:
                        ot_new = opool.tile([128, gw], bf16,
                                            tag=f"ot{gi % 3}_{m % 2}")
                        otiles[(gi, m)] = ot_new
                    ot = otiles[(gi, m)]
                    dst = ot[:, col - gcol:col - gcol + width]
                    if (ci + m) % 2 == 0:
                        nc.scalar.copy(dst, ps)
                    else:
                        nc.vector.tensor_copy(dst, ps)
                    if ci == last_ci:
                        nc.sync.dma_start(
                            out=out[m * 128:(m + 1) * 128, gcol:gcol + gw],
                            in_=ot)
    nc.finalize()
    return nc


def _install_neff_cache():
    """Persistent NEFF cache for the bass_exec compile path.

    concourse's neuronx_cc_hook compiles the embedded BIR with walrus on
    every fresh process (60-190s) and never consults libneuronxla's NEFF
    cache. The BIR built by _build_bass_matmul is byte-deterministic, so a
    content-addressed on-disk cache makes every process after the first
    skip the compile entirely.
    """
    import hashlib
    import os
    import shutil
    import concourse.bass2jax as b2j
    if getattr(b2j, "_bass_neff_cache_installed", False):
        return
    orig = b2j.compile_bir_kernel
    cache_root = os.environ.get(
        "BASS_NEFF_CACHE_DIR",
        os.path.expanduser("~/.neuron-compile-cache/bass-neff"))

    def cached_compile(bir_json, tmpdir, neff_name="file.neff"):
        try:
            raw = bir_json if isinstance(bir_json, bytes) else bir_json.encode()
            key = hashlib.sha256(raw).hexdigest()
            cpath = os.path.join(cache_root, key + ".neff")
            if os.path.exists(cpath):
                dst = os.path.join(tmpdir, neff_name)
                shutil.copyfile(cpath, dst)
                return dst
            neff_path = orig(bir_json, tmpdir, neff_name=neff_name)
            os.makedirs(cache_root, exist_ok=True)
            tmp = cpath + ".tmp"
            shutil.copyfile(neff_path, tmp)
            os.replace(tmp, cpath)
            return neff_path
        except Exception:
            return orig(bir_json, tmpdir, neff_name=neff_name)

    b2j.compile_bir_kernel = cached_compile
    b2j._bass_neff_cache_installed = True


def _readout_device(pre_flat, wT_bf16):
    """pre_flat [1024, 512] fp32 -> logits [1024, 32000] via 8-core bass."""
    import os
    from concourse.bass_utils import run_bass_kernel_spmd
    _install_neff_cache()
    if "nc" not in _BASS_CACHE:
        _BASS_CACHE["nc"] = _build_bass_matmul()
    nc = _BASS_CACHE["nc"]
    preT = np.ascontiguousarray(pre_flat.T).astype(_BF16)    # [512, 1024] bf16
    in_maps = [
        {"preT": preT, "wT": np.ascontiguousarray(wT_bf16[:, k * VSH:(k + 1) * VSH])}
        for k in range(N_CORES)
    ]
    core_ids = list(range(N_CORES))
    try:
        res = run_bass_kernel_spmd(nc, in_maps, core_ids=core_ids)
    except Exception:
        # e.g. BASS_TRACE set but the axon NTFF profile hook is not
        # available in this container: retry with tracing disabled
        os.environ["BASS_NEVER_TRACE"] = "1"
        res = run_bass_kernel_spmd(nc, in_maps, core_ids=core_ids)
    _BASS_CACHE["last_exec_ns"] = res.exec_time_ns
    _BASS_CACHE["last_results"] = res
    return np.concatenate(
        [r["out"].astype(np.float32) for r in res.results], axis=1)


def kernel(x_enc, x_enc_k, h0, c0, x_mask, y_train, word_emb, W_ih, W_hh,
           b_ih, b_hh, w_trg_W, w_trg_b, w_att_W, w_att_b, ctx2r_W, readout_W):
    x_enc = np.asarray(x_enc, np.float32)
    x_enc_k = np.asarray(x_enc_k, np.float32)
    y_train = np.asarray(y_train)
    B, Ly = y_train.shape
    pre_all = _recurrence(x_enc, x_enc_k, np.asarray(h0), np.asarray(c0),
                          np.asarray(x_mask), y_train, np.asarray(word_emb),
                          np.asarray(W_ih), np.asarray(W_hh), np.asarray(b_ih),
                          np.asarray(b_hh), np.asarray(w_trg_W),
                          np.asarray(w_trg_b), np.asarray(w_att_W),
                          np.asarray(w_att_b), np.asarray(ctx2r_W))
    pre_flat = pre_all.reshape(Ly * B, D)                # [1024, 512]
    wT = np.ascontiguousarray(np.asarray(readout_W, np.float32).T)  # [512, V]
    try:
        if _BF16 is None:
            raise RuntimeError("ml_dtypes unavailable")
        logits_flat = _readout_device(pre_flat, wT.astype(_BF16))
        # spot-check a few token rows against the fp32 host matmul; a
        # healthy bf16 device result sits at ~4e-3 relative error, so 1.5e-2
        # catches any transient bad execution well before the 2e-2 gate
        idx = [0, 401, 702, 1023]
        ref_rows = pre_flat[idx] @ wT
        spot = np.abs(logits_flat[idx] - ref_rows).max() / max(
            np.abs(ref_rows).max(), 1e-12)
        if not np.isfinite(spot) or spot > 1.5e-2:
            raise RuntimeError(f"device spot-check failed (rel {spot:.3e})")
    except Exception as exc:                             # robust fallback
        import traceback
        traceback.print_exc()
        print(f"[kernel] device readout failed ({exc!r}); numpy fallback")
        _BASS_CACHE.pop("last_exec_ns", None)
        _BASS_CACHE.pop("last_results", None)
        logits_flat = pre_flat @ wT
    logits = logits_flat.reshape(Ly, B, V)
    return np.swapaxes(logits, 0, 1).astype(np.float32)  # [B, Ly, V]
